# revision 14
# baseline (speedup 1.0000x reference)
"""HSTU attention (B=2, L=2048, D=1024, H=16) on 8 TRN2 NeuronCores.

Sharding: batch (2) x head-group (4 heads, 256 features) -> 8 cores.

Per core, for its batch b and 4 heads:
  - Projections run as 3-term fp8 DoubleRow matmuls: x and 16*W are sent as
    fp8 (hi) plus fp8 residual (lo); psum accumulates hi*hi + hi*lo + lo*hi
    (the dropped lo*lo term is ~1e-3 relative).  0.75x the cycles of bf16
    at bf16-class accuracy; the 1/16 is folded into the psum->SBUF copies.
  - Scores S^T = K^T.T @ Q in bf16, [keys x queries] layout, psum tiles of
    [128, 1024] (2 banks); exp(S/8) on ACT (scale=0.125) -> bf16 e tiles.
  - Key chunks beyond max(seq_len) are skipped (runtime-specialized NJ);
    masking is folded into the AV operands: V is premasked into vF (valid)
    and vP (prompt&valid), true-diagonal 128x128 blocks get a {0,1} mask
    multiply (Pool engine), row sums use mask columns.
  - AV is swapped: out[tokens, feats] += e_chunk.T @ v (N=64), with N=1
    row-sum matmuls into a shared psum bank; softmax normalization + U
    gating is a per-partition scalar_tensor_tensor from an SBUF copy.
  - g is transposed per 128x128 chunk: DMA xbar transpose for the first
    half (ec0, mid-kernel), PE transpose via identity for the tail half.
  - W_o partials per ec-half in bf16; outputs land in two bf16 partial
    tensors, DMA'd four token-chunks at a time.
Host sums the 8 partial outputs per batch.

Scheduling: a software-pipelined (jc, query-half) unit loop per head with
hooks spreading projections / W_o groups into PE slack; per-chunk SBUF
tiles avoid false tile-granularity dependencies; a warm-up matmul chain
brings the PE out of its low p-state during the initial DMA window.
"""

import sys

for _p in ("/opt/trn_rl_repo", "/root/.axon_site/_ro/trn_rl_repo"):
    if _p not in sys.path:
        sys.path.insert(0, _p)

import numpy as np
import ml_dtypes

import concourse.bass as bass  # noqa: F401
import concourse.mybir as mybir
import concourse.tile as tile
from concourse import bacc
from concourse.bass_utils import run_bass_kernel_spmd

F32 = mybir.dt.float32
BF16 = mybir.dt.bfloat16
F8 = mybir.dt.float8e4
EXP = mybir.ActivationFunctionType.Exp
COPY = mybir.ActivationFunctionType.Copy
DR = mybir.MatmulPerfMode.DoubleRow
MULT = mybir.AluOpType.mult

B, L, D, H = 2, 2048, 1024, 16
DK = D // H          # 64
HPC = 4              # heads per core
E = HPC * DK         # 256 features per core
NDC = D // 128       # 8 contraction chunks for projections
NLC = L // 128       # 16 token chunks
NIC = L // 512       # 4 token 512-spans

_cache = {}


def _build(NJ):
    NLK = NJ * 128
    kspans = [(s, min(512, NLK - s)) for s in range(0, NLK, 512)]

    nc = bacc.Bacc("TRN2", target_bir_lowering=False, debug=False)

    xd = {
        t: nc.dram_tensor(f"x_{t}", [128, NDC, L], F8, kind="ExternalInput").ap()
        for t in ("h", "l")
    }
    wd = {
        (nm, t): nc.dram_tensor(f"w{nm}_{t}", [128, NDC, E], F8, kind="ExternalInput").ap()
        for nm in ("q", "k", "v", "u") for t in ("h", "l")
    }
    wo16d = nc.dram_tensor("wo16", [128, 2, D], BF16, kind="ExternalInput").ap()
    dm16d = nc.dram_tensor("dm16", [128, NJ, 128], BF16, kind="ExternalInput").ap()
    mc16d = nc.dram_tensor("mc16", [128, NJ, 3], BF16, kind="ExternalInput").ap()
    mxfd = nc.dram_tensor("mxf", [128, NJ, 2], F32, kind="ExternalInput").ap()
    identd = nc.dram_tensor("ident", [128, 128], BF16, kind="ExternalInput").ap()
    outd = [
        nc.dram_tensor(f"out{ec}", [L, D], BF16, kind="ExternalOutput").ap()
        for ec in range(2)
    ]
    # out viewed as [tok-in-chunk 128, chunk 16, feat 1024] for merged DMAs
    outr = [o.rearrange("(a p) d -> p a d", p=128) for o in outd]

    with tile.TileContext(nc) as tc:
        with tc.tile_pool(name="persist", bufs=1) as persist, \
             tc.tile_pool(name="e8p", bufs=5) as e8p, \
             tc.tile_pool(name="eDp", bufs=5) as eDp, \
             tc.tile_pool(name="osb", bufs=2) as osb:
            xs = {
                (s, t): persist.tile([128, NDC, 512], F8, tag=f"xs{s}{t}", name=f"xs{s}{t}")
                for s in range(NIC) for t in ("h", "l")
            }
            w8 = {
                k: persist.tile([128, NDC, E], F8, tag=f"w{k[0]}{k[1]}", name=f"w{k[0]}{k[1]}")
                for k in wd
            }
            wo16 = persist.tile([128, 2, D], BF16, tag="wo16", name="wo16")
            dm16 = persist.tile([128, NJ, 128], BF16, tag="dm16", name="dm16")
            mc16 = persist.tile([128, NJ, 3], BF16, tag="mc16", name="mc16")
            mxf = persist.tile([128, NJ, 2], F32, tag="mxf", name="mxf")
            ident = persist.tile([128, 128], BF16, tag="ident", name="ident")
            wtmp = persist.tile([128, 512], BF16, tag="wtmp", name="wtmp")
            q16 = [persist.tile([128, L], BF16, tag=f"q16_{ec}", name=f"q16_{ec}")
                   for ec in range(2)]
            k16 = [persist.tile([128, NLK], BF16, tag=f"k16_{ec}", name=f"k16_{ec}")
                   for ec in range(2)]
            u16 = [persist.tile([128, E], BF16, tag=f"u16_{lc}", name=f"u16_{lc}")
                   for lc in range(NLC)]
            vF8 = [persist.tile([128, E], BF16, tag=f"vF_{jc}", name=f"vF_{jc}")
                   for jc in range(NJ)]
            vP8 = [persist.tile([128, E], BF16, tag=f"vP_{jc}", name=f"vP_{jc}")
                   for jc in range(NJ)]
            g16 = [persist.tile([128, E], BF16, tag=f"g_{lc}", name=f"g_{lc}")
                   for lc in range(NLC)]
            gT16 = {(ec, lc): persist.tile([128, 128], BF16, tag=f"gt{ec}_{lc}", name=f"gt{ec}_{lc}")
                    for ec in range(2) for lc in range(NLC)}
            avs = persist.tile([128, 1024], F32, tag="avs", name="avs")
            rec16 = [persist.tile([128, 16], F32, tag=f"rec{p}", name=f"rec{p}")
                     for p in range(2)]

            # -------- emission helpers --------
            def dma_x(si, which=("h", "l")):
                s0 = si * 512
                for t in which:
                    nc.sync.dma_start(out=xs[(si, t)], in_=xd[t][:, :, s0 : s0 + 512])

            def proj_mms(p, w, lhs_of, rhs_of):
                """3-term hi/lo DR accumulation into psum slice p[:, 0:w]."""
                terms = (("h", "h"), ("h", "l"), ("l", "h"))
                n = NDC // 2
                first = True
                for (tx, tw) in terms:
                    for t in range(n):
                        nc.tensor.matmul(
                            p[:, 0:w],
                            lhs_of(tx, tw, t),
                            rhs_of(tx, tw, t),
                            start=first,
                            stop=(tx, tw) == ("l", "h") and t == n - 1,
                            perf_mode=DR,
                        )
                        first = False

            def proj_qk(pool, nm, ec, c0, w):
                """q16/k16[ec][:, c0:c0+w] = (x @ (16W).T)/16 in [feat, tok]."""
                p = pool.tile([128, 512], F32, tag="pp", name="pp")
                si, o = c0 // 512, c0 % 512
                proj_mms(
                    p, w,
                    lambda tx, tw, t: w8[(nm, tw)][:, 2 * t : 2 * t + 2, ec * 128 : (ec + 1) * 128],
                    lambda tx, tw, t: xs[(si, tx)][:, 2 * t : 2 * t + 2, o : o + w],
                )
                dest = q16 if nm == "q" else k16
                with nc.allow_low_precision(reason="bf16 store"):
                    nc.vector.tensor_scalar_mul(
                        dest[ec][:, c0 : c0 + w], p[:, 0:w], 1.0 / 16.0
                    )

            def proj_v(pool, h, jc):
                hsl = slice(64 * h, 64 * h + 64)
                si, o = (jc * 128) // 512, (jc * 128) % 512
                p = pool.tile([128, 512], F32, tag="pp", name="pp")
                proj_mms(
                    p, 64,
                    lambda tx, tw, t: xs[(si, tx)][:, 2 * t : 2 * t + 2, o : o + 128],
                    lambda tx, tw, t: w8[("v", tw)][:, 2 * t : 2 * t + 2, hsl],
                )
                with nc.allow_low_precision(reason="bf16 store"):
                    nc.vector.tensor_scalar_mul(
                        vF8[jc][:, hsl], p[:, 0:64], mxf[:, jc, 0:1]
                    )
                    nc.vector.tensor_scalar_mul(
                        vP8[jc][:, hsl], p[:, 0:64], mxf[:, jc, 1:2]
                    )

            def proj_u(pool, h, lc):
                hsl = slice(64 * h, 64 * h + 64)
                si, o = (lc * 128) // 512, (lc * 128) % 512
                p = pool.tile([128, 512], F32, tag="pp", name="pp")
                proj_mms(
                    p, 64,
                    lambda tx, tw, t: xs[(si, tx)][:, 2 * t : 2 * t + 2, o : o + 128],
                    lambda tx, tw, t: w8[("u", tw)][:, 2 * t : 2 * t + 2, hsl],
                )
                with nc.allow_low_precision(reason="bf16 store"):
                    nc.vector.tensor_scalar_mul(
                        u16[lc][:, hsl], p[:, 0:64], 1.0 / 16.0
                    )

            def scores_exp(scp, h, jc, half):
                """e tile [128 keys, 1024 queries] = exp(S/8) for (h, jc, half).
                Also precomputes the diag-masked eD tile when this (jc, half)
                contains the true-diagonal block."""
                ec, hh = h // 2, h % 2
                jsl = slice(jc * 128, (jc + 1) * 128)
                e8 = e8p.tile([128, 1024], BF16, tag="e8", name="e8")
                sc = scp.tile([128, 1024], F32, tag="sc", name="sc")
                for q in range(2):
                    q0 = half * 1024 + q * 512
                    nc.tensor.matmul(
                        sc[:, q * 512 : (q + 1) * 512],
                        k16[ec][64 * hh : 64 * hh + 64, jsl],
                        q16[ec][64 * hh : 64 * hh + 64, q0 : q0 + 512],
                        start=True, stop=True,
                    )
                with nc.allow_low_precision(reason="bf16 exp"):
                    nc.scalar.activation(e8, sc, EXP, scale=0.125)
                eD = None
                if half * 8 <= jc < half * 8 + 8:
                    eD = eDp.tile([128, 128], BF16, tag="eD", name="eD")
                    loc = jc * 128 - half * 1024
                    with nc.allow_low_precision(reason="mask mul"):
                        nc.gpsimd.tensor_mul(
                            eD, e8[:, loc : loc + 128], dm16[:, jc, :]
                        )
                return e8, eD

            def av_half(av, rs, h, jc, half, e8, eD):
                hsl = slice(64 * h, 64 * h + 64)
                base = half * 8
                for lc in range(base, base + 8):
                    loc = lc * 128 - half * 1024
                    if jc == lc:
                        lhsT, vt, mcol = eD, vF8, 2
                    elif jc < lc:
                        lhsT, vt, mcol = e8[:, loc : loc + 128], vF8, 0
                    else:
                        lhsT, vt, mcol = e8[:, loc : loc + 128], vP8, 1
                    nc.tensor.matmul(
                        av[:, lc * 64 : (lc + 1) * 64],
                        lhsT, vt[jc][:, hsl],
                        start=(jc == 0 and lc == base),
                        stop=(jc == NJ - 1 and lc == base + 7),
                    )
                    nc.tensor.matmul(
                        rs[:, (h % 2) * 16 + lc : (h % 2) * 16 + lc + 1],
                        lhsT, mc16[:, jc, mcol : mcol + 1],
                        start=(jc == 0 and half == 0 and lc == 0),
                        stop=(jc == NJ - 1 and lc == NLC - 1),
                    )

            def head_att(scp, projp, av, rs, h, pre=(), hooks=None):
                hooks = hooks or {}
                pend = []
                ui = 0
                for half in range(2):
                    for jc in range(NJ):
                        for f in hooks.get(ui, ()):
                            f()
                        e, eD = scores_exp(scp, h, jc, half)
                        if half == 0:
                            proj_v(projp, h, jc)
                        if ui == 0:
                            for f in pre:
                                f()
                        if len(pend) >= 3:
                            av_half(av, rs, h, *pend.pop(0))
                        pend.append((jc, half, e, eD))
                        ui += 1
                for item in pend:
                    av_half(av, rs, h, *item)

            def gate(av, rs, h):
                p = h % 2
                with nc.allow_low_precision(reason="gate"):
                    nc.vector.reciprocal(rec16[p], rs[:, p * 16 : (p + 1) * 16])
                    nc.vector.tensor_copy(avs, av)
                    for lc in range(NLC):
                        nc.vector.scalar_tensor_tensor(
                            g16[lc][:, 64 * h : 64 * h + 64],
                            avs[:, lc * 64 : (lc + 1) * 64],
                            rec16[p][:, lc : lc + 1],
                            u16[lc][:, 64 * h : 64 * h + 64],
                            MULT, MULT,
                        )

            def transposes_dma(ec):
                for lc in range(NLC):
                    nc.sync.dma_start_transpose(
                        gT16[(ec, lc)],
                        g16[lc][:, ec * 128 : (ec + 1) * 128],
                    )

            wo_alt = [0]
            osb_cur = [None]

            def wo_step(wop, ec, lc, fc, tail=False):
                """one W_o matmul + copy; every 8th step fires the quad DMA."""
                q, s = lc // 4, lc % 4
                if osb_cur[0] is None:
                    osb_cur[0] = osb.tile([128, 4, 1024], BF16, tag="osb", name="osb")
                o = osb_cur[0]
                p = wop.tile([128, 512], F32, tag="pp", name="pp")
                nc.tensor.matmul(
                    p,
                    gT16[(ec, lc)],
                    wo16[:, ec, fc * 512 : (fc + 1) * 512],
                    start=True, stop=True,
                )
                wo_alt[0] += 1
                with nc.allow_low_precision(reason="bf16 out"):
                    if tail and wo_alt[0] % 2 == 0:
                        nc.scalar.activation(
                            o[:, s, fc * 512 : (fc + 1) * 512], p, COPY
                        )
                    else:
                        nc.vector.tensor_copy(
                            o[:, s, fc * 512 : (fc + 1) * 512], p
                        )
                if s == 3 and fc == 1:
                    nc.sync.dma_start(
                        out=outr[ec][:, 4 * q : 4 * q + 4, :], in_=o
                    )
                    osb_cur[0] = None

            NU = 2 * NJ  # units per head

            def spread(jobs, lo, hi):
                """jobs: list of (cost, fn); place by cumulative cost."""
                hooks = {}
                total = sum(c for c, _ in jobs) or 1
                acc = 0
                for c, job in jobs:
                    hooks.setdefault(lo + (acc * (hi - lo)) // total, []).append(job)
                    acc += c
                return hooks

            with tc.tile_pool(name="av", bufs=1, space="PSUM") as avp, \
                 tc.tile_pool(name="rs", bufs=1, space="PSUM") as rsp:
                av = avp.tile([128, 1024], F32, tag="av", name="av")
                rs = rsp.tile([128, 32], F32, tag="rs", name="rs")

                # -------- phase 1: warmup, DMAs, h0, QK proj, U(h0) --------
                with tc.tile_pool(name="pp", bufs=3, space="PSUM") as pp, \
                     tc.tile_pool(name="sc1", bufs=1, space="PSUM") as sc1:
                    # PE warm-up chain during the initial DMA window
                    nc.vector.memset(wtmp, 0.0)
                    wp = pp.tile([128, 512], F32, tag="pp", name="pp")
                    for i in range(5):
                        nc.tensor.matmul(
                            wp, wtmp[:, 0:128], wtmp,
                            start=(i == 0), stop=(i == 4),
                        )

                    # input DMAs (x on SP queue, weights/masks on ACT queue)
                    dma_x(0, ("h",))
                    nc.scalar.dma_start(out=w8[("k", "h")], in_=wd[("k", "h")])
                    nc.scalar.dma_start(out=w8[("q", "h")], in_=wd[("q", "h")])
                    dma_x(1, ("h",))
                    dma_x(0, ("l",))
                    nc.scalar.dma_start(out=w8[("k", "l")], in_=wd[("k", "l")])
                    nc.scalar.dma_start(out=w8[("q", "l")], in_=wd[("q", "l")])
                    dma_x(1, ("l",))
                    for t in ("h", "l"):
                        nc.scalar.dma_start(out=w8[("v", t)], in_=wd[("v", t)])
                    for t in ("h", "l"):
                        nc.scalar.dma_start(out=w8[("u", t)], in_=wd[("u", t)])
                    nc.scalar.dma_start(out=dm16, in_=dm16d)
                    nc.scalar.dma_start(out=mc16, in_=mc16d)
                    nc.scalar.dma_start(out=mxf, in_=mxfd)
                    nc.scalar.dma_start(out=wo16, in_=wo16d)
                    nc.scalar.dma_start(out=ident, in_=identd)

                    proj_qk(pp, "k", 0, 0, 512)
                    proj_qk(pp, "q", 0, 0, 512)
                    proj_qk(pp, "q", 0, 512, 512)

                    jobs0 = []
                    jobs0.append((1, lambda: dma_x(2)))
                    for (c0, w) in kspans[1:2]:
                        jobs0.append((3, lambda c0=c0, w=w: proj_qk(pp, "k", 0, c0, w)))
                    jobs0.append((3, lambda: proj_qk(pp, "q", 0, 1024, 512)))
                    jobs0.append((1, lambda: dma_x(3)))
                    for (c0, w) in kspans[2:]:
                        jobs0.append((3, lambda c0=c0, w=w: proj_qk(pp, "k", 0, c0, w)))
                    jobs0.append((3, lambda: proj_qk(pp, "q", 0, 1536, 512)))
                    for lc in range(NLC):
                        jobs0.append((1, lambda lc=lc: proj_u(pp, 0, lc)))
                    for (c0, w) in kspans:
                        jobs0.append((3, lambda c0=c0, w=w: proj_qk(pp, "k", 1, c0, w)))
                    for ic in range(NIC):
                        jobs0.append((3, lambda ic=ic: proj_qk(pp, "q", 1, ic * 512, 512)))
                    head_att(sc1, pp, av, rs, 0, hooks=spread(jobs0, 1, NU))

                # -------- phase 2: h1-h3, ec0 wo --------
                with tc.tile_pool(name="sc2", bufs=2, space="PSUM") as sc2, \
                     tc.tile_pool(name="wop", bufs=1, space="PSUM") as wop:
                    jobs1 = [(1, lambda lc=lc: proj_u(wop, 1, lc)) for lc in range(NLC)]
                    head_att(sc2, wop, av, rs, 1,
                             pre=[lambda: gate(av, rs, 0)],
                             hooks=spread(jobs1, 1, NU))

                    jobs2 = [(1, lambda lc=lc: proj_u(wop, 2, lc)) for lc in range(NLC)]
                    jobs2 += [(1, lambda lc=lc, fc=fc: wo_step(wop, 0, lc, fc))
                              for lc in range(8) for fc in range(2)]
                    head_att(sc2, wop, av, rs, 2,
                             pre=[lambda: gate(av, rs, 1), lambda: transposes_dma(0)],
                             hooks=spread(jobs2, 1, NU))

                    jobs3 = [(1, lambda lc=lc: proj_u(wop, 3, lc)) for lc in range(NLC)]
                    jobs3 += [(1, lambda lc=lc, fc=fc: wo_step(wop, 0, lc, fc))
                              for lc in range(8, NLC) for fc in range(2)]
                    head_att(sc2, wop, av, rs, 3,
                             pre=[lambda: gate(av, rs, 2)],
                             hooks=spread(jobs3, 1, NU))
                    gate(av, rs, 3)

                    # ---- tail: ec1 transposes + wo inside the same scope ----
                    def tail_wo(lc):
                        q, s = lc // 4, lc % 4
                        if osb_cur[0] is None:
                            osb_cur[0] = osb.tile([128, 4, 1024], BF16, tag="osb", name="osb")
                        o = osb_cur[0]
                        p = sc2.tile([128, 1024], F32, tag="sc", name="sc")
                        for fc in range(2):
                            nc.tensor.matmul(
                                p[:, fc * 512 : (fc + 1) * 512],
                                gT16[(1, lc)],
                                wo16[:, 1, fc * 512 : (fc + 1) * 512],
                                start=True, stop=True,
                            )
                        with nc.allow_low_precision(reason="bf16 out"):
                            if lc % 2 == 0:
                                nc.scalar.activation(o[:, s, :], p, COPY)
                            else:
                                nc.vector.tensor_copy(o[:, s, :], p)
                        if s == 3:
                            nc.sync.dma_start(
                                out=outr[1][:, 4 * q : 4 * q + 4, :], in_=o
                            )
                            osb_cur[0] = None

                    nc.scalar.dma_start_transpose(
                        gT16[(1, 0)], g16[0][:, 128:256]
                    )
                    nc.scalar.dma_start_transpose(
                        gT16[(1, 1)], g16[1][:, 128:256]
                    )
                    for lc in range(NLC):
                        if lc + 2 < NLC:
                            nc.scalar.dma_start_transpose(
                                gT16[(1, lc + 2)], g16[lc + 2][:, 128:256]
                            )
                        tail_wo(lc)

    nc.compile()
    return nc


def _hilo(a):
    f8 = ml_dtypes.float8_e4m3
    hi = a.astype(f8)
    lo = (a - hi.astype(np.float32)).astype(f8)
    return hi, lo


def _host_inputs(NJ, x, token_types, seq_lens, W_q, W_k, W_v, W_u, W_o):
    x = np.asarray(x, dtype=np.float32)
    token_types = np.asarray(token_types)
    seq_lens = np.asarray(seq_lens)
    W = {
        "q": np.asarray(W_q, dtype=np.float32),
        "k": np.asarray(W_k, dtype=np.float32),
        "v": np.asarray(W_v, dtype=np.float32),
        "u": np.asarray(W_u, dtype=np.float32),
    }
    W_o = np.asarray(W_o, dtype=np.float32)
    bf = ml_dtypes.bfloat16

    per_batch = []
    for b in range(B):
        xt = np.ascontiguousarray(x[b].T.reshape(NDC, 128, L).transpose(1, 0, 2))
        xh, xl = _hilo(xt)
        prompt = np.asarray(token_types[b] < 3)
        valid = np.arange(L) < int(seq_lens[b])
        dm16 = np.zeros((128, NJ, 128), bf)
        mc16 = np.zeros((128, NJ, 3), bf)
        mxf = np.zeros((128, NJ, 2), np.float32)
        for jc in range(NJ):
            j = np.arange(jc * 128, (jc + 1) * 128)
            i = j  # true-diagonal block
            allow = valid[j][:, None] & (prompt[j][:, None] | (j[:, None] <= i[None, :]))
            dm16[:, jc, :] = allow.astype(np.float32)
            mF = valid[j].astype(np.float32)
            mP = (valid[j] & prompt[j]).astype(np.float32)
            mc16[:, jc, 0] = mF
            mc16[:, jc, 1] = mP
            mc16[:, jc, 2] = 1.0
            mxf[:, jc, 0] = mF / 16.0
            mxf[:, jc, 1] = mP / 16.0
        per_batch.append((xh, xl, dm16, mc16, mxf))

    ident = np.eye(128, dtype=bf)
    in_maps = []
    for c in range(8):
        b, gi = c // 4, c % 4
        e0 = E * gi
        xh, xl, dm16, mc16, mxf = per_batch[b]
        m = {"x_h": xh, "x_l": xl, "dm16": dm16, "mc16": mc16, "mxf": mxf,
             "ident": ident}
        for nm in ("q", "k", "v", "u"):
            wt = np.ascontiguousarray(
                (W[nm][e0 : e0 + E] * 16.0).T.reshape(NDC, 128, E).transpose(1, 0, 2)
            )
            wh, wl = _hilo(wt)
            m[f"w{nm}_h"] = wh
            m[f"w{nm}_l"] = wl
        m["wo16"] = np.ascontiguousarray(
            W_o[:, e0 : e0 + E].T.reshape(2, 128, D).transpose(1, 0, 2)
        ).astype(bf)
        in_maps.append(m)
    return in_maps


def kernel(x, token_types, seq_lens, W_q, W_k, W_v, W_u, W_o, **_run_kwargs):
    seq = np.asarray(seq_lens)
    NJ = int(np.ceil(seq.max() / 128.0))
    NJ = max(1, min(NLC, NJ))
    if ("nc", NJ) not in _cache:
        _cache[("nc", NJ)] = _build(NJ)
    nc = _cache[("nc", NJ)]
    in_maps = _host_inputs(NJ, x, token_types, seq_lens, W_q, W_k, W_v, W_u, W_o)
    try:
        res = run_bass_kernel_spmd(nc, in_maps, list(range(8)), **_run_kwargs)
    except Exception as ex:  # transient NRT device wedge: retry once
        if "UNRECOVERABLE" not in str(ex) and "UNAVAILABLE" not in str(ex):
            raise
        res = run_bass_kernel_spmd(nc, in_maps, list(range(8)), **_run_kwargs)
    _cache["last_result"] = res
    _cache["nc"] = nc  # for test.py TimelineSim
    full = np.zeros((B, L, D), np.float64)
    for c in range(8):
        r = res.results[c]
        full[c // 4] += r["out0"].astype(np.float64) + r["out1"].astype(np.float64)
    return full.astype(np.float32)


# revision 15
# speedup vs baseline: 1.0523x; 1.0523x over previous
"""HSTU attention (B=2, L=2048, D=1024, H=16) on 8 TRN2 NeuronCores.

Sharding: batch (2) x head-group (4 heads, 256 features) -> 8 cores.

Per core, for its batch b and 4 heads:
  - Projections run as 3-term fp8 DoubleRow matmuls: x and 16*W are sent as
    fp8 (hi) plus fp8 residual (lo); psum accumulates hi*hi + hi*lo + lo*hi
    (the dropped lo*lo term is ~1e-3 relative).  0.75x the cycles of bf16
    at bf16-class accuracy; the 1/16 is folded into the psum->SBUF copies.
  - Scores S^T = K^T.T @ Q in bf16, [keys x queries] layout, psum tiles of
    [128, 1024] (2 banks); exp(S/8) on ACT (scale=0.125) -> bf16 e tiles.
  - Key chunks beyond max(seq_len) are skipped (runtime-specialized NJ);
    masking is folded into the AV operands: V is premasked into vF (valid)
    and vP (prompt&valid), true-diagonal 128x128 blocks get a {0,1} mask
    multiply (Pool engine), row sums use mask columns.
  - AV is swapped: out[tokens, feats] += e_chunk.T @ v (N=64), with N=1
    row-sum matmuls into a shared psum bank; softmax normalization + U
    gating is a per-partition scalar_tensor_tensor from an SBUF copy.
  - g is transposed per 128x128 chunk: DMA xbar transpose for the first
    half (ec0, mid-kernel), PE transpose via identity for the tail half.
  - W_o partials per ec-half in bf16; outputs land in two bf16 partial
    tensors, DMA'd four token-chunks at a time.
Host sums the 8 partial outputs per batch.

Scheduling: a software-pipelined (jc, query-half) unit loop per head with
hooks spreading projections / W_o groups into PE slack; per-chunk SBUF
tiles avoid false tile-granularity dependencies; a warm-up matmul chain
brings the PE out of its low p-state during the initial DMA window.
"""

import sys

for _p in ("/opt/trn_rl_repo", "/root/.axon_site/_ro/trn_rl_repo"):
    if _p not in sys.path:
        sys.path.insert(0, _p)

import numpy as np
import ml_dtypes

import concourse.bass as bass  # noqa: F401
import concourse.mybir as mybir
import concourse.tile as tile
from concourse import bacc
from concourse.bass_utils import run_bass_kernel_spmd

F32 = mybir.dt.float32
BF16 = mybir.dt.bfloat16
F8 = mybir.dt.float8e4
EXP = mybir.ActivationFunctionType.Exp
COPY = mybir.ActivationFunctionType.Copy
DR = mybir.MatmulPerfMode.DoubleRow
MULT = mybir.AluOpType.mult

B, L, D, H = 2, 2048, 1024, 16
DK = D // H          # 64
HPC = 4              # heads per core
E = HPC * DK         # 256 features per core
NDC = D // 128       # 8 contraction chunks for projections
NLC = L // 128       # 16 token chunks
NIC = L // 512       # 4 token 512-spans

_cache = {}


def _build(NJ):
    NLK = NJ * 128
    kspans = [(s, min(512, NLK - s)) for s in range(0, NLK, 512)]

    nc = bacc.Bacc("TRN2", target_bir_lowering=False, debug=False)

    xd = {
        t: nc.dram_tensor(f"x_{t}", [128, NDC, L], F8, kind="ExternalInput").ap()
        for t in ("h", "l")
    }
    wd = {
        (nm, t): nc.dram_tensor(f"w{nm}_{t}", [128, NDC, E], F8, kind="ExternalInput").ap()
        for nm in ("q", "k", "v", "u") for t in ("h", "l")
    }
    wo16d = nc.dram_tensor("wo16", [128, 2, D], BF16, kind="ExternalInput").ap()
    dm16d = nc.dram_tensor("dm16", [128, NJ, 128], BF16, kind="ExternalInput").ap()
    mc16d = nc.dram_tensor("mc16", [128, NJ, 3], BF16, kind="ExternalInput").ap()
    mxfd = nc.dram_tensor("mxf", [128, NJ, 2], F32, kind="ExternalInput").ap()
    identd = nc.dram_tensor("ident", [128, 128], BF16, kind="ExternalInput").ap()
    outd = [
        nc.dram_tensor(f"out{ec}", [L, D], BF16, kind="ExternalOutput").ap()
        for ec in range(2)
    ]
    # out viewed as [tok-in-chunk 128, chunk 16, feat 1024] for merged DMAs
    outr = [o.rearrange("(a p) d -> p a d", p=128) for o in outd]

    with tile.TileContext(nc) as tc:
        with tc.tile_pool(name="persist", bufs=1) as persist, \
             tc.tile_pool(name="e8p", bufs=5) as e8p, \
             tc.tile_pool(name="eDp", bufs=5) as eDp, \
             tc.tile_pool(name="osb", bufs=2) as osb:
            xs = {
                (s, t): persist.tile([128, NDC, 512], F8, tag=f"xs{s}{t}", name=f"xs{s}{t}")
                for s in range(NIC) for t in ("h", "l")
            }
            w8 = {
                k: persist.tile([128, NDC, E], F8, tag=f"w{k[0]}{k[1]}", name=f"w{k[0]}{k[1]}")
                for k in wd
            }
            wo16 = persist.tile([128, 2, D], BF16, tag="wo16", name="wo16")
            dm16 = persist.tile([128, NJ, 128], BF16, tag="dm16", name="dm16")
            mc16 = persist.tile([128, NJ, 3], BF16, tag="mc16", name="mc16")
            mxf = persist.tile([128, NJ, 2], F32, tag="mxf", name="mxf")
            ident = persist.tile([128, 128], BF16, tag="ident", name="ident")
            wtmp = persist.tile([128, 512], BF16, tag="wtmp", name="wtmp")
            q16 = [persist.tile([128, L], BF16, tag=f"q16_{ec}", name=f"q16_{ec}")
                   for ec in range(2)]
            k16 = [persist.tile([128, NLK], BF16, tag=f"k16_{ec}", name=f"k16_{ec}")
                   for ec in range(2)]
            u16 = [persist.tile([128, E], BF16, tag=f"u16_{lc}", name=f"u16_{lc}")
                   for lc in range(NLC)]
            vF8 = [persist.tile([128, E], BF16, tag=f"vF_{jc}", name=f"vF_{jc}")
                   for jc in range(NJ)]
            vP8 = [persist.tile([128, E], BF16, tag=f"vP_{jc}", name=f"vP_{jc}")
                   for jc in range(NJ)]
            g16 = [persist.tile([128, E], BF16, tag=f"g_{lc}", name=f"g_{lc}")
                   for lc in range(NLC)]
            gT16 = {(ec, lc): persist.tile([128, 128], BF16, tag=f"gt{ec}_{lc}", name=f"gt{ec}_{lc}")
                    for ec in range(2) for lc in range(NLC)}
            avs = persist.tile([128, 1024], F32, tag="avs", name="avs")
            rec16 = [persist.tile([128, 16], F32, tag=f"rec{p}", name=f"rec{p}")
                     for p in range(2)]

            # -------- emission helpers --------
            def dma_x(si, which=("h", "l")):
                s0 = si * 512
                for t in which:
                    nc.sync.dma_start(out=xs[(si, t)], in_=xd[t][:, :, s0 : s0 + 512])

            def proj_mms(p, w, lhs_of, rhs_of):
                """3-term hi/lo DR accumulation into psum slice p[:, 0:w]."""
                terms = (("h", "h"), ("h", "l"), ("l", "h"))
                n = NDC // 2
                first = True
                for (tx, tw) in terms:
                    for t in range(n):
                        nc.tensor.matmul(
                            p[:, 0:w],
                            lhs_of(tx, tw, t),
                            rhs_of(tx, tw, t),
                            start=first,
                            stop=(tx, tw) == ("l", "h") and t == n - 1,
                            perf_mode=DR,
                        )
                        first = False

            def proj_qk(pool, nm, ec, c0, w):
                """q16/k16[ec][:, c0:c0+w] = (x @ (16W).T)/16 in [feat, tok]."""
                p = pool.tile([128, 512], F32, tag="pp", name="pp")
                si, o = c0 // 512, c0 % 512
                proj_mms(
                    p, w,
                    lambda tx, tw, t: w8[(nm, tw)][:, 2 * t : 2 * t + 2, ec * 128 : (ec + 1) * 128],
                    lambda tx, tw, t: xs[(si, tx)][:, 2 * t : 2 * t + 2, o : o + w],
                )
                dest = q16 if nm == "q" else k16
                with nc.allow_low_precision(reason="bf16 store"):
                    nc.vector.tensor_scalar_mul(
                        dest[ec][:, c0 : c0 + w], p[:, 0:w], 1.0 / 16.0
                    )

            def proj_v(pool, h, jc):
                hsl = slice(64 * h, 64 * h + 64)
                si, o = (jc * 128) // 512, (jc * 128) % 512
                p = pool.tile([128, 512], F32, tag="pp", name="pp")
                proj_mms(
                    p, 64,
                    lambda tx, tw, t: xs[(si, tx)][:, 2 * t : 2 * t + 2, o : o + 128],
                    lambda tx, tw, t: w8[("v", tw)][:, 2 * t : 2 * t + 2, hsl],
                )
                with nc.allow_low_precision(reason="bf16 store"):
                    nc.vector.tensor_scalar_mul(
                        vF8[jc][:, hsl], p[:, 0:64], mxf[:, jc, 0:1]
                    )
                    nc.vector.tensor_scalar_mul(
                        vP8[jc][:, hsl], p[:, 0:64], mxf[:, jc, 1:2]
                    )

            def proj_u(pool, h, lc):
                hsl = slice(64 * h, 64 * h + 64)
                si, o = (lc * 128) // 512, (lc * 128) % 512
                p = pool.tile([128, 512], F32, tag="pp", name="pp")
                proj_mms(
                    p, 64,
                    lambda tx, tw, t: xs[(si, tx)][:, 2 * t : 2 * t + 2, o : o + 128],
                    lambda tx, tw, t: w8[("u", tw)][:, 2 * t : 2 * t + 2, hsl],
                )
                with nc.allow_low_precision(reason="bf16 store"):
                    nc.vector.tensor_scalar_mul(
                        u16[lc][:, hsl], p[:, 0:64], 1.0 / 16.0
                    )

            def scores_exp(scp, h, jc, half):
                """e tile [128 keys, 1024 queries] = exp(S/8) for (h, jc, half).
                Also precomputes the diag-masked eD tile when this (jc, half)
                contains the true-diagonal block."""
                ec, hh = h // 2, h % 2
                jsl = slice(jc * 128, (jc + 1) * 128)
                e8 = e8p.tile([128, 1024], BF16, tag="e8", name="e8")
                sc = scp.tile([128, 1024], F32, tag="sc", name="sc")
                for q in range(2):
                    q0 = half * 1024 + q * 512
                    nc.tensor.matmul(
                        sc[:, q * 512 : (q + 1) * 512],
                        k16[ec][64 * hh : 64 * hh + 64, jsl],
                        q16[ec][64 * hh : 64 * hh + 64, q0 : q0 + 512],
                        start=True, stop=True,
                    )
                with nc.allow_low_precision(reason="bf16 exp"):
                    nc.scalar.activation(e8, sc, EXP, scale=0.125)
                eD = None
                if half * 8 <= jc < half * 8 + 8:
                    eD = eDp.tile([128, 128], BF16, tag="eD", name="eD")
                    loc = jc * 128 - half * 1024
                    with nc.allow_low_precision(reason="mask mul"):
                        nc.gpsimd.tensor_mul(
                            eD, e8[:, loc : loc + 128], dm16[:, jc, :]
                        )
                return e8, eD

            def av_half(av, rs, h, jc, half, e8, eD):
                hsl = slice(64 * h, 64 * h + 64)
                base = half * 8
                for lc in range(base, base + 8):
                    loc = lc * 128 - half * 1024
                    if jc == lc:
                        lhsT, vt, mcol = eD, vF8, 2
                    elif jc < lc:
                        lhsT, vt, mcol = e8[:, loc : loc + 128], vF8, 0
                    else:
                        lhsT, vt, mcol = e8[:, loc : loc + 128], vP8, 1
                    nc.tensor.matmul(
                        av[:, lc * 64 : (lc + 1) * 64],
                        lhsT, vt[jc][:, hsl],
                        start=(jc == 0 and lc == base),
                        stop=(jc == NJ - 1 and lc == base + 7),
                    )
                    nc.tensor.matmul(
                        rs[:, (h % 2) * 16 + lc : (h % 2) * 16 + lc + 1],
                        lhsT, mc16[:, jc, mcol : mcol + 1],
                        start=(jc == 0 and half == 0 and lc == 0),
                        stop=(jc == NJ - 1 and lc == NLC - 1),
                    )

            def head_att(scp, projp, av, rs, h, pre=(), hooks=None):
                hooks = hooks or {}
                pend = []
                ui = 0
                for half in range(2):
                    for jc in range(NJ):
                        for f in hooks.get(ui, ()):
                            f()
                        e, eD = scores_exp(scp, h, jc, half)
                        if half == 0:
                            proj_v(projp, h, jc)
                        if ui == 0:
                            for f in pre:
                                f()
                        if len(pend) >= 3:
                            av_half(av, rs, h, *pend.pop(0))
                        pend.append((jc, half, e, eD))
                        ui += 1
                for item in pend:
                    av_half(av, rs, h, *item)

            def gate(av, rs, h):
                p = h % 2
                with nc.allow_low_precision(reason="gate"):
                    nc.vector.reciprocal(rec16[p], rs[:, p * 16 : (p + 1) * 16])
                    nc.vector.tensor_copy(avs, av)
                    for lc in range(NLC):
                        nc.vector.scalar_tensor_tensor(
                            g16[lc][:, 64 * h : 64 * h + 64],
                            avs[:, lc * 64 : (lc + 1) * 64],
                            rec16[p][:, lc : lc + 1],
                            u16[lc][:, 64 * h : 64 * h + 64],
                            MULT, MULT,
                        )

            def transposes_dma(ec):
                for lc in range(NLC):
                    nc.sync.dma_start_transpose(
                        gT16[(ec, lc)],
                        g16[lc][:, ec * 128 : (ec + 1) * 128],
                    )

            wo_alt = [0]
            osb_cur = [None]

            def wo_step(wop, ec, lc, fc, tail=False):
                """one W_o matmul + copy; every 8th step fires the quad DMA."""
                q, s = lc // 4, lc % 4
                if osb_cur[0] is None:
                    osb_cur[0] = osb.tile([128, 4, 1024], BF16, tag="osb", name="osb")
                o = osb_cur[0]
                p = wop.tile([128, 512], F32, tag="pp", name="pp")
                nc.tensor.matmul(
                    p,
                    gT16[(ec, lc)],
                    wo16[:, ec, fc * 512 : (fc + 1) * 512],
                    start=True, stop=True,
                )
                wo_alt[0] += 1
                with nc.allow_low_precision(reason="bf16 out"):
                    if tail and wo_alt[0] % 2 == 0:
                        nc.scalar.activation(
                            o[:, s, fc * 512 : (fc + 1) * 512], p, COPY
                        )
                    else:
                        nc.vector.tensor_copy(
                            o[:, s, fc * 512 : (fc + 1) * 512], p
                        )
                if s == 3 and fc == 1:
                    nc.sync.dma_start(
                        out=outr[ec][:, 4 * q : 4 * q + 4, :], in_=o
                    )
                    osb_cur[0] = None

            NU = 2 * NJ  # units per head

            def spread(jobs, lo, hi):
                """jobs: list of (cost, fn); place by cumulative cost."""
                hooks = {}
                total = sum(c for c, _ in jobs) or 1
                acc = 0
                for c, job in jobs:
                    hooks.setdefault(lo + (acc * (hi - lo)) // total, []).append(job)
                    acc += c
                return hooks

            with tc.tile_pool(name="av", bufs=1, space="PSUM") as avp, \
                 tc.tile_pool(name="rs", bufs=1, space="PSUM") as rsp:
                av = avp.tile([128, 1024], F32, tag="av", name="av")
                rs = rsp.tile([128, 32], F32, tag="rs", name="rs")

                # -------- phase 1: warmup, DMAs, h0, QK proj, U(h0) --------
                with tc.tile_pool(name="pp", bufs=3, space="PSUM") as pp, \
                     tc.tile_pool(name="sc1", bufs=1, space="PSUM") as sc1:
                    # PE warm-up chain during the initial DMA window
                    nc.vector.memset(wtmp, 0.0)
                    wp = pp.tile([128, 512], F32, tag="pp", name="pp")
                    for i in range(5):
                        nc.tensor.matmul(
                            wp, wtmp[:, 0:128], wtmp,
                            start=(i == 0), stop=(i == 4),
                        )

                    # input DMAs (x on SP queue, weights/masks on ACT queue)
                    dma_x(0, ("h",))
                    nc.scalar.dma_start(out=w8[("k", "h")], in_=wd[("k", "h")])
                    nc.scalar.dma_start(out=w8[("q", "h")], in_=wd[("q", "h")])
                    dma_x(1, ("h",))
                    dma_x(0, ("l",))
                    nc.scalar.dma_start(out=w8[("k", "l")], in_=wd[("k", "l")])
                    nc.scalar.dma_start(out=w8[("q", "l")], in_=wd[("q", "l")])
                    dma_x(1, ("l",))
                    for t in ("h", "l"):
                        nc.scalar.dma_start(out=w8[("v", t)], in_=wd[("v", t)])
                    for t in ("h", "l"):
                        nc.scalar.dma_start(out=w8[("u", t)], in_=wd[("u", t)])
                    nc.scalar.dma_start(out=dm16, in_=dm16d)
                    nc.scalar.dma_start(out=mc16, in_=mc16d)
                    nc.scalar.dma_start(out=mxf, in_=mxfd)
                    nc.scalar.dma_start(out=wo16, in_=wo16d)
                    nc.scalar.dma_start(out=ident, in_=identd)

                    proj_qk(pp, "k", 0, 0, 512)
                    proj_qk(pp, "q", 0, 0, 512)
                    proj_qk(pp, "q", 0, 512, 512)

                    jobs0 = []
                    jobs0.append((1, lambda: dma_x(2)))
                    for (c0, w) in kspans[1:2]:
                        jobs0.append((3, lambda c0=c0, w=w: proj_qk(pp, "k", 0, c0, w)))
                    jobs0.append((3, lambda: proj_qk(pp, "q", 0, 1024, 512)))
                    jobs0.append((1, lambda: dma_x(3)))
                    for (c0, w) in kspans[2:]:
                        jobs0.append((3, lambda c0=c0, w=w: proj_qk(pp, "k", 0, c0, w)))
                    jobs0.append((3, lambda: proj_qk(pp, "q", 0, 1536, 512)))
                    for lc in range(NLC):
                        jobs0.append((1, lambda lc=lc: proj_u(pp, 0, lc)))
                    for (c0, w) in kspans:
                        jobs0.append((3, lambda c0=c0, w=w: proj_qk(pp, "k", 1, c0, w)))
                    for ic in range(NIC):
                        jobs0.append((3, lambda ic=ic: proj_qk(pp, "q", 1, ic * 512, 512)))
                    head_att(sc1, pp, av, rs, 0, hooks=spread(jobs0, 1, NU))

                # -------- phase 2: h1-h3, ec0 wo --------
                with tc.tile_pool(name="sc2", bufs=2, space="PSUM") as sc2, \
                     tc.tile_pool(name="wop", bufs=1, space="PSUM") as wop:
                    jobs1 = [(1, lambda lc=lc: proj_u(wop, 1, lc)) for lc in range(NLC)]
                    head_att(sc2, wop, av, rs, 1,
                             pre=[lambda: gate(av, rs, 0)],
                             hooks=spread(jobs1, 1, NU))

                    jobs2 = [(1, lambda lc=lc: proj_u(wop, 2, lc)) for lc in range(NLC)]
                    jobs2 += [(1, lambda lc=lc, fc=fc: wo_step(wop, 0, lc, fc))
                              for lc in range(8) for fc in range(2)]
                    head_att(sc2, wop, av, rs, 2,
                             pre=[lambda: gate(av, rs, 1), lambda: transposes_dma(0)],
                             hooks=spread(jobs2, 1, NU))

                    jobs3 = [(1, lambda lc=lc: proj_u(wop, 3, lc)) for lc in range(NLC)]
                    jobs3 += [(1, lambda lc=lc, fc=fc: wo_step(wop, 0, lc, fc))
                              for lc in range(8, NLC) for fc in range(2)]
                    head_att(sc2, wop, av, rs, 3,
                             pre=[lambda: gate(av, rs, 2)],
                             hooks=spread(jobs3, 1, NU))
                    gate(av, rs, 3)

            # -------- phase 3: tail: PE transposes + ec1 wo --------
            with tc.tile_pool(name="wo2", bufs=3, space="PSUM") as wo2, \
                 tc.tile_pool(name="tp", bufs=2, space="PSUM") as tpp:
                def tail_tp(lc):
                    t = tpp.tile([128, 128], BF16, tag="tp", name="tp")
                    nc.tensor.transpose(t, g16[lc][:, 128:256], ident)
                    with nc.allow_low_precision(reason="bf16 transpose"):
                        if lc % 2 == 0:
                            nc.vector.tensor_copy(gT16[(1, lc)], t)
                        else:
                            nc.scalar.activation(gT16[(1, lc)], t, COPY)

                def tail_wo(lc):
                    q, s = lc // 4, lc % 4
                    if osb_cur[0] is None:
                        osb_cur[0] = osb.tile([128, 4, 1024], BF16, tag="osb", name="osb")
                    o = osb_cur[0]
                    p = wo2.tile([128, 1024], F32, tag="wq", name="wq")
                    for fc in range(2):
                        nc.tensor.matmul(
                            p[:, fc * 512 : (fc + 1) * 512],
                            gT16[(1, lc)],
                            wo16[:, 1, fc * 512 : (fc + 1) * 512],
                            start=True, stop=True,
                        )
                    with nc.allow_low_precision(reason="bf16 out"):
                        if lc % 2 == 0:
                            nc.scalar.activation(o[:, s, :], p, COPY)
                        else:
                            nc.vector.tensor_copy(o[:, s, :], p)
                    if s == 3:
                        nc.sync.dma_start(
                            out=outr[1][:, 4 * q : 4 * q + 4, :], in_=o
                        )
                        osb_cur[0] = None

                tail_tp(0)
                tail_tp(1)
                for lc in range(NLC):
                    if lc + 2 < NLC:
                        tail_tp(lc + 2)
                    tail_wo(lc)

    nc.compile()
    return nc


def _hilo(a):
    f8 = ml_dtypes.float8_e4m3
    hi = a.astype(f8)
    lo = (a - hi.astype(np.float32)).astype(f8)
    return hi, lo


def _host_inputs(NJ, x, token_types, seq_lens, W_q, W_k, W_v, W_u, W_o):
    x = np.asarray(x, dtype=np.float32)
    token_types = np.asarray(token_types)
    seq_lens = np.asarray(seq_lens)
    W = {
        "q": np.asarray(W_q, dtype=np.float32),
        "k": np.asarray(W_k, dtype=np.float32),
        "v": np.asarray(W_v, dtype=np.float32),
        "u": np.asarray(W_u, dtype=np.float32),
    }
    W_o = np.asarray(W_o, dtype=np.float32)
    bf = ml_dtypes.bfloat16

    per_batch = []
    for b in range(B):
        xt = np.ascontiguousarray(x[b].T.reshape(NDC, 128, L).transpose(1, 0, 2))
        xh, xl = _hilo(xt)
        prompt = np.asarray(token_types[b] < 3)
        valid = np.arange(L) < int(seq_lens[b])
        dm16 = np.zeros((128, NJ, 128), bf)
        mc16 = np.zeros((128, NJ, 3), bf)
        mxf = np.zeros((128, NJ, 2), np.float32)
        for jc in range(NJ):
            j = np.arange(jc * 128, (jc + 1) * 128)
            i = j  # true-diagonal block
            allow = valid[j][:, None] & (prompt[j][:, None] | (j[:, None] <= i[None, :]))
            dm16[:, jc, :] = allow.astype(np.float32)
            mF = valid[j].astype(np.float32)
            mP = (valid[j] & prompt[j]).astype(np.float32)
            mc16[:, jc, 0] = mF
            mc16[:, jc, 1] = mP
            mc16[:, jc, 2] = 1.0
            mxf[:, jc, 0] = mF / 16.0
            mxf[:, jc, 1] = mP / 16.0
        per_batch.append((xh, xl, dm16, mc16, mxf))

    ident = np.eye(128, dtype=bf)
    in_maps = []
    for c in range(8):
        b, gi = c // 4, c % 4
        e0 = E * gi
        xh, xl, dm16, mc16, mxf = per_batch[b]
        m = {"x_h": xh, "x_l": xl, "dm16": dm16, "mc16": mc16, "mxf": mxf,
             "ident": ident}
        for nm in ("q", "k", "v", "u"):
            wt = np.ascontiguousarray(
                (W[nm][e0 : e0 + E] * 16.0).T.reshape(NDC, 128, E).transpose(1, 0, 2)
            )
            wh, wl = _hilo(wt)
            m[f"w{nm}_h"] = wh
            m[f"w{nm}_l"] = wl
        m["wo16"] = np.ascontiguousarray(
            W_o[:, e0 : e0 + E].T.reshape(2, 128, D).transpose(1, 0, 2)
        ).astype(bf)
        in_maps.append(m)
    return in_maps


def kernel(x, token_types, seq_lens, W_q, W_k, W_v, W_u, W_o, **_run_kwargs):
    seq = np.asarray(seq_lens)
    NJ = int(np.ceil(seq.max() / 128.0))
    NJ = max(1, min(NLC, NJ))
    if ("nc", NJ) not in _cache:
        _cache[("nc", NJ)] = _build(NJ)
    nc = _cache[("nc", NJ)]
    in_maps = _host_inputs(NJ, x, token_types, seq_lens, W_q, W_k, W_v, W_u, W_o)
    try:
        res = run_bass_kernel_spmd(nc, in_maps, list(range(8)), **_run_kwargs)
    except Exception as ex:  # transient NRT device wedge: retry once
        if "UNRECOVERABLE" not in str(ex) and "UNAVAILABLE" not in str(ex):
            raise
        res = run_bass_kernel_spmd(nc, in_maps, list(range(8)), **_run_kwargs)
    _cache["last_result"] = res
    _cache["nc"] = nc  # for test.py TimelineSim
    full = np.zeros((B, L, D), np.float64)
    for c in range(8):
        r = res.results[c]
        full[c // 4] += r["out0"].astype(np.float64) + r["out1"].astype(np.float64)
    return full.astype(np.float32)
